# revision 34
# baseline (speedup 1.0000x reference)
"""ContrastiveHead loss kernel for 8 Trainium2 NeuronCores — v3.

Data-parallel shard of B across 8 cores; each core MLPs its 1024 rows
(transposed layout, fp8 DoubleRow for the two hidden layers), normalizes
the [E=128, 1024] features, all-gathers bf16 features, then computes its
[1024, 8192] sim block and the masked logsumexp.

v3 structure:
- Weights pinned in SBUF (loaded once, spread across DMA queues early).
- The MLP/norm runs in two 512-row halves; each half's normalized
  features are all-gathered immediately, so the first gather overlaps
  the second half's compute and only the second gather's tail is
  exposed.
- Sim phase: [128, 2048] PSUM chunks; 2/3 of chunks exponentiate on the
  Scalar engine (Exp with accum_out producing the row-sum directly);
  1/3 on the Vector engine via a Schraudolph integer exp (bias constant
  tuned so the loss error stays ~1e-5), keeping both engines busy.
- logsumexp via the constant bound max=1: lse = 1/T + log(sum_j
  exp((S_ij-1)/T)); self term subtracted via locally recomputed S_ii;
  pos diagonals from the local block-gram with the partner tile.
"""

import os
import sys

for _p in ("/opt/trn_rl_repo",):
    if os.path.isdir(_p) and _p not in sys.path:
        sys.path.append(_p)

import ml_dtypes
import numpy as np

import concourse.bass as bass
import concourse.mybir as mybir
import concourse.tile as tile
from concourse import bacc
from concourse.bass_utils import run_bass_kernel_spmd
from concourse.masks import make_identity

BF16 = ml_dtypes.bfloat16
F32 = mybir.dt.float32
I32 = mybir.dt.int32
BF = mybir.dt.bfloat16
F8 = mybir.dt.float8e4
FP8 = mybir.dt.np(F8)

B, D, H, E = 4096, 2048, 2048, 128
T = 0.07
SCALE = float(1.0 / T)
NCORES = 8
BS = B // NCORES          # rows per view per core (512)
M = 2 * BS                # local feature rows (1024)
HM = M // 2               # rows per pipeline half (512)
KT = D // 128             # 16 contraction tiles for D/H
NT = H // 128             # 16 output-feature tiles for hidden layers
MT = M // 128             # 8 local row tiles
NG = NCORES * M           # 8192 gathered rows
CHUNK = 2048              # sim free-dim chunk (4-bank PSUM tile)
NCHUNK = NG // CHUNK      # 4 sim chunks per row tile

# Schraudolph integer exp, int16/bf16 variant: the bf16 bit pattern of
# exp(x) is approximately int16(A16*x + B16); the bf16 output gets the DVE
# 2x reduce rate. B16 offset tuned for minimal loss bias.
SCH_A16 = float(2.0**23 / np.log(2.0) / 65536.0)
SCH_MUL = SCH_A16 * SCALE
SCH_ADD = float((16256.0 - 4.0) - SCH_A16 * SCALE)
SCHRAU = os.environ.get("KERNEL_SCHRAU", "1") == "1"

SKIP = set(os.environ.get("KERNEL_SKIP", "").split(",")) - {""}


def _build():
    nc = bacc.Bacc(num_devices=NCORES)

    x = nc.dram_tensor("x", [128, KT, M], F8, kind="ExternalInput")
    w0 = nc.dram_tensor("w0", [NT, 128, KT, 128], F8, kind="ExternalInput")
    w1 = nc.dram_tensor("w1", [NT, 128, KT, 128], F8, kind="ExternalInput")
    w2 = nc.dram_tensor("w2", [128, KT, 128], BF, kind="ExternalInput")
    b0 = nc.dram_tensor("b0", [128, NT], F32, kind="ExternalInput")
    b1 = nc.dram_tensor("b1", [128, NT], F32, kind="ExternalInput")
    b2 = nc.dram_tensor("b2", [128, 1], F32, kind="ExternalInput")
    # raw per-row stats [stot | dself | dpos]; the cheap final
    # log/exp/combine runs on the host in f64
    out = nc.dram_tensor("out", [128, 3 * MT], F32, kind="ExternalOutput")

    AF = mybir.ActivationFunctionType
    MULT = mybir.AluOpType.mult
    ADD = mybir.AluOpType.add
    DR = mybir.MatmulPerfMode.DoubleRow

    with tile.TileContext(nc) as tc:
        with (
            tc.tile_pool(name="singles", bufs=1) as singles,
            tc.tile_pool(name="small", bufs=4) as small,
            tc.tile_pool(name="esc", bufs=3) as esc,
            tc.tile_pool(name="pmm", bufs=2, space="PSUM") as pmm,
            tc.tile_pool(name="dram", bufs=1, space="DRAM") as dram,
        ):
            iss = [nc.sync, nc.scalar, nc.gpsimd]

            # ---- pinned weight slabs; first L0 slabs lead the queue ----
            w0s = []
            for tn in range(NT):
                ws = singles.tile([128, KT, 128], F8, name=f"w0s{tn}")
                w0s.append(ws)
            w1s = []
            for tn in range(NT):
                ws = singles.tile([128, KT, 128], F8, name=f"w1s{tn}")
                w1s.append(ws)

            # issue weight/input DMAs in first-use order, round-robin across
            # the three posting queues: the first L0 chain blocks on w0s[0] +
            # x[k0,k1]; w0s[tn] is needed at ~23+4.3*(tn//4) us; w1s[tn] at
            # ~41+1.3*tn us
            a_x = singles.tile([128, KT, M], F8)
            wsl2 = singles.tile([128, KT, 128], BF)
            order = [("w0", 0), ("x", 0), ("x", 1)]
            order += [("x", tk) for tk in range(2, 6)]
            order += [("w0", 1)]
            order += [("x", tk) for tk in range(6, 10)]
            order += [("w0", 2)]
            order += [("x", tk) for tk in range(10, 14)]
            order += [("w0", 3), ("x", 14), ("x", 15)]
            order += [("w0", tn) for tn in range(4, NT)]
            order += [("w1", tn) for tn in range(NT)]
            order += [("w2", 0)]
            for i, (kind, j) in enumerate(order):
                eng = iss[i % 3]
                if kind == "x":
                    eng.dma_start(out=a_x[:, j, :], in_=x[:, j, :])
                elif kind == "w0":
                    eng.dma_start(out=w0s[j], in_=w0[j])
                elif kind == "w1":
                    eng.dma_start(out=w1s[j], in_=w1[j])
                else:
                    eng.dma_start(out=wsl2, in_=w2[:, :, :])

            ident = singles.tile([128, 128], F32)
            make_identity(nc, ident)
            b0s = singles.tile([128, NT], F32)
            b1s = singles.tile([128, NT], F32)
            b2s = singles.tile([128, 1], F32)
            nc.scalar.dma_start(out=b0s, in_=b0[:, :])
            nc.scalar.dma_start(out=b1s, in_=b1[:, :])
            nc.scalar.dma_start(out=b2s, in_=b2[:, :])
            ones = singles.tile([128, 128], F32)
            nc.vector.memset(ones, 1.0)
            nbias = singles.tile([128, 1], F32)
            nc.vector.memset(nbias, -SCALE)
            pbias = singles.tile([128, 1], F32)
            nc.vector.memset(pbias, SCALE)

            h0 = singles.tile([128, NT, M], F8)
            h1 = singles.tile([128, NT, M], BF)
            eT = singles.tile([128, M], F32)
            sq = singles.tile([128, M], F32)
            rnorm = singles.tile([128, M], F32)
            rrec = singles.tile([128, M], F32)
            fT = singles.tile([128, M], BF)
            FT = singles.tile([128, NG], BF)

            def hidden_layer(src, dst, weights, bias_s, func, hsl):
                """fp8 DoubleRow layer on one 512-row half; 4 tn chains per
                PSUM tile, 4 512-col ACT drains."""
                for tn0 in range(0, NT, 4):
                    ps = pmm.tile([128, CHUNK], F32, tag="mm")
                    for j in range(4):
                        tn = tn0 + j
                        for tk in range(0, KT, 2):
                            nc.tensor.matmul(
                                ps[:, j * 512 : (j + 1) * 512],
                                lhsT=weights[tn][:, tk : tk + 2, :],
                                rhs=src[:, tk : tk + 2, hsl],
                                start=(tk == 0),
                                stop=(tk == KT - 2),
                                perf_mode=DR,
                            )
                    for j in range(4):
                        tn = tn0 + j
                        nc.scalar.activation(
                            out=dst[:, tn, hsl],
                            in_=ps[:, j * 512 : (j + 1) * 512],
                            func=func,
                            bias=bias_s[:, tn : tn + 1],
                            scale=1.0,
                        )

            def l2_norm_gather(csl, ft_off, tag):
                """Layer 2 + normalize + all-gather one row-group; the
                gathered blocks land at FT[:, ft_off:ft_off+8*width]."""
                width = csl.stop - csl.start
                ps2 = pmm.tile([128, CHUNK], F32, tag="mm")
                for tk in range(KT):
                    nc.tensor.matmul(
                        ps2[:, 0:width],
                        lhsT=wsl2[:, tk, :],
                        rhs=h1[:, tk, csl],
                        start=(tk == 0),
                        stop=(tk == KT - 1),
                    )
                nc.scalar.activation(
                    out=eT[:, csl], in_=ps2[:, 0:width], func=AF.Identity,
                    bias=b2s[:, 0:1], scale=1.0,
                )
                nc.vector.tensor_mul(sq[:, csl], eT[:, csl], eT[:, csl])
                nc.tensor.matmul(
                    ps2[:, width : 2 * width], lhsT=ones, rhs=sq[:, csl],
                    start=True, stop=True,
                )
                nc.scalar.activation(
                    out=rnorm[:, csl], in_=ps2[:, width : 2 * width], func=AF.Sqrt,
                    scale=1.0,
                )
                nc.vector.reciprocal_approx_fast(
                    out=rrec[:, csl], in_=rnorm[:, csl]
                )
                nc.vector.tensor_mul(fT[:, csl], eT[:, csl], rrec[:, csl])

                cc_in = dram.tile([128, width], BF, name=f"cc_in{tag}")
                cc_out = dram.tile([NCORES * 128, width], BF, name=f"cc_out{tag}")
                nc.sync.dma_start(out=cc_in, in_=fT[:, csl])
                if "collective" in SKIP:
                    for r in range(NCORES):
                        nc.sync.dma_start(
                            out=cc_out[r * 128 : (r + 1) * 128, :], in_=cc_in[:, :]
                        )
                else:
                    nc.gpsimd.collective_compute(
                        "AllGather",
                        mybir.AluOpType.bypass,
                        replica_groups=[list(range(NCORES))],
                        ins=[cc_in.opt()],
                        outs=[cc_out.opt()],
                    )
                nc.sync.dma_start(
                    out=FT[:, ft_off : ft_off + NCORES * width],
                    in_=cc_out[:, :].rearrange("(r p) w -> p r w", r=NCORES),
                )

            # FT layout: [half-A blocks (4096) | Q3 blocks (2048) | Q4 (2048)]
            # Half B gathers as two quarters so the two mesh collectives
            # overlap and the final one carries only 64KB/core.
            # Both L0 halves run before L1 so the first w1 slab isn't needed
            # until ~75us into the HBM-limited weight stream.
            hidden_layer(a_x, h0, w0s, b0s, AF.Relu, slice(0, HM))
            hidden_layer(a_x, h0, w0s, b0s, AF.Relu, slice(HM, M))
            hidden_layer(h0, h1, w1s, b1s, AF.Identity, slice(0, HM))
            l2_norm_gather(slice(0, HM), 0, "A")
            hidden_layer(h0, h1, w1s, b1s, AF.Identity, slice(HM, M))
            l2_norm_gather(slice(HM, HM + HM // 2), NG // 2, "Q3")
            l2_norm_gather(slice(HM + HM // 2, M), 3 * NG // 4, "Q4")

            # ---- self/pos diagonals from local features ----
            dself_all = singles.tile([128, MT], F32)
            dpos_all = singles.tile([128, MT], F32)
            for m in range(MT):
                pm = (m + MT // 2) % MT
                lhs = fT[:, m * 128 : (m + 1) * 128]
                psd = pmm.tile([128, CHUNK], F32, tag="mm")
                nc.tensor.matmul(
                    psd[:, 0:128], lhsT=lhs, rhs=fT[:, m * 128 : (m + 1) * 128],
                    start=True, stop=True,
                )
                nc.tensor.matmul(
                    psd[:, 128:256], lhsT=lhs, rhs=fT[:, pm * 128 : (pm + 1) * 128],
                    start=True, stop=True,
                )
                dsc = small.tile([128, 128], F32, tag="dscratch")
                nc.vector.tensor_mul(dsc, psd[:, 0:128], ident)
                nc.vector.reduce_sum(
                    dself_all[:, m : m + 1], dsc, axis=mybir.AxisListType.X
                )
                dsc2 = small.tile([128, 128], F32, tag="dscratch")
                nc.vector.tensor_mul(dsc2, psd[:, 128:256], ident)
                nc.vector.reduce_sum(
                    dpos_all[:, m : m + 1], dsc2, axis=mybir.AxisListType.X
                )

            # ship the diagonals while the sim phase runs
            nc.scalar.dma_start(out=out[:, MT : 2 * MT], in_=dself_all)
            nc.scalar.dma_start(out=out[:, 2 * MT : 3 * MT], in_=dpos_all)

            # ---- sim + exp-sum; Exp+accum on ACT, Schraudolph on DVE ----
            outv = singles.tile([128, MT], F32)
            stot_all = singles.tile([128, MT], F32)
            sums = singles.tile([128, MT, NCHUNK], F32)
            if "phase3" in SKIP:
                nc.vector.tensor_copy(outv, fT[:, :MT])
            # c-outer order: chunks over the first-half columns (c 0,1)
            # depend only on the first gather and fill the second gather's
            # latency window.
            for c, m in ([] if "phase3" in SKIP else
                         [(c, m) for c in range(NCHUNK) for m in range(MT)]):
                idx = c * MT + m
                lhs = fT[:, m * 128 : (m + 1) * 128]
                ps = pmm.tile([128, CHUNK], F32, tag="mm")
                for q in range(CHUNK // 512):
                    j0 = c * CHUNK + q * 512
                    nc.tensor.matmul(
                        ps[:, q * 512 : (q + 1) * 512],
                        lhsT=lhs, rhs=FT[:, j0 : j0 + 512],
                        start=True, stop=True,
                    )
                if SCHRAU and idx % 8 in (2, 5, 7):
                    sch = esc.tile([128, CHUNK], mybir.dt.int16, tag="sch")
                    nc.vector.tensor_scalar(
                        out=sch, in0=ps, scalar1=SCH_MUL, scalar2=SCH_ADD,
                        op0=MULT, op1=ADD,
                    )
                    nc.vector.reduce_sum(
                        sums[:, m, c : c + 1], sch.bitcast(BF),
                        axis=mybir.AxisListType.X,
                    )
                else:
                    escr = esc.tile([128, CHUNK], BF, tag="escr")
                    nc.scalar.activation(
                        out=escr, in_=ps, func=AF.Exp, scale=SCALE,
                        bias=nbias, accum_out=sums[:, m, c : c + 1],
                    )
            if "phase3" not in SKIP:
                nc.vector.reduce_sum(
                    stot_all, sums, axis=mybir.AxisListType.X
                )

            if "phase3" in SKIP:
                nc.sync.dma_start(out=out[:, 0:MT], in_=outv)
            else:
                nc.sync.dma_start(out=out[:, 0:MT], in_=stot_all)

    nc.finalize()
    return nc


_NC_CACHE = None


def _get_nc():
    global _NC_CACHE
    if _NC_CACHE is None:
        _NC_CACHE = _build()
    return _NC_CACHE


def host_reduce(o):
    """Sum of per-row losses from one core's raw [stot | dself | dpos] out."""
    o = np.asarray(o, np.float64)
    stot, dself, dpos = o[:, 0:MT], o[:, MT : 2 * MT], o[:, 2 * MT : 3 * MT]
    sexcl = stot - np.exp(SCALE * dself - SCALE)
    return (np.log(sexcl) + SCALE * (1.0 - dpos)).sum()


def _prep_w(W, ntiles, dt=BF16):
    K = W.shape[0]
    kt = K // 128
    arr = W.reshape(kt, 128, ntiles, 128).transpose(2, 1, 0, 3)
    return np.ascontiguousarray(arr.astype(dt))


def _prep_b(b, ntiles):
    return np.ascontiguousarray(
        np.asarray(b, np.float32).reshape(ntiles, 128).T
    )


def kernel(input1, input2, W0, b0, W1, b1, W2, b2):
    input1 = np.asarray(input1, np.float32)
    input2 = np.asarray(input2, np.float32)
    w0p = _prep_w(np.asarray(W0, np.float32), NT, FP8)
    w1p = _prep_w(np.asarray(W1, np.float32), NT, FP8)
    w2p = _prep_w(np.asarray(W2, np.float32), 1)[0]
    b0p = _prep_b(b0, NT)
    b1p = _prep_b(b1, NT)
    b2p = np.ascontiguousarray(np.asarray(b2, np.float32).reshape(128, 1))

    in_maps = []
    for r in range(NCORES):
        xr = np.concatenate(
            [input1[r * BS : (r + 1) * BS], input2[r * BS : (r + 1) * BS]], axis=0
        )
        xp = np.ascontiguousarray(
            xr.reshape(M, KT, 128).transpose(2, 1, 0).astype(FP8)
        )
        in_maps.append(
            {
                "x": xp, "w0": w0p, "w1": w1p, "w2": w2p,
                "b0": b0p, "b1": b1p, "b2": b2p,
            }
        )

    nc = _get_nc()
    res = run_bass_kernel_spmd(
        nc,
        in_maps,
        core_ids=list(range(NCORES)),
        trace=bool(int(os.environ.get("KERNEL_TRACE", "0"))),
    )
    total = np.float64(0.0)
    for r in range(NCORES):
        total += host_reduce(np.asarray(res.results[r]["out"], np.float64))
    loss = np.float32(total / (2 * B))
    if res.exec_time_ns is not None:
        kernel.last_exec_time_ns = res.exec_time_ns
    return np.asarray(loss, np.float32)


kernel.last_exec_time_ns = None


# revision 35
# speedup vs baseline: 1.0581x; 1.0581x over previous
"""ContrastiveHead loss kernel for 8 Trainium2 NeuronCores — v3.

Data-parallel shard of B across 8 cores; each core MLPs its 1024 rows
(transposed layout, fp8 DoubleRow for the two hidden layers), normalizes
the [E=128, 1024] features, all-gathers bf16 features, then computes its
[1024, 8192] sim block and the masked logsumexp.

v3 structure:
- Weights pinned in SBUF (loaded once, spread across DMA queues early).
- The MLP/norm runs in two 512-row halves; each half's normalized
  features are all-gathered immediately, so the first gather overlaps
  the second half's compute and only the second gather's tail is
  exposed.
- Sim phase: [128, 2048] PSUM chunks; 2/3 of chunks exponentiate on the
  Scalar engine (Exp with accum_out producing the row-sum directly);
  1/3 on the Vector engine via a Schraudolph integer exp (bias constant
  tuned so the loss error stays ~1e-5), keeping both engines busy.
- logsumexp via the constant bound max=1: lse = 1/T + log(sum_j
  exp((S_ij-1)/T)); self term subtracted via locally recomputed S_ii;
  pos diagonals from the local block-gram with the partner tile.
"""

import os
import sys

for _p in ("/opt/trn_rl_repo",):
    if os.path.isdir(_p) and _p not in sys.path:
        sys.path.append(_p)

import ml_dtypes
import numpy as np

import concourse.bass as bass
import concourse.mybir as mybir
import concourse.tile as tile
from concourse import bacc
from concourse.bass_utils import run_bass_kernel_spmd
from concourse.masks import make_identity

BF16 = ml_dtypes.bfloat16
F32 = mybir.dt.float32
I32 = mybir.dt.int32
BF = mybir.dt.bfloat16
F8 = mybir.dt.float8e4
FP8 = mybir.dt.np(F8)

B, D, H, E = 4096, 2048, 2048, 128
T = 0.07
SCALE = float(1.0 / T)
NCORES = 8
BS = B // NCORES          # rows per view per core (512)
M = 2 * BS                # local feature rows (1024)
HM = M // 2               # rows per pipeline half (512)
KT = D // 128             # 16 contraction tiles for D/H
NT = H // 128             # 16 output-feature tiles for hidden layers
MT = M // 128             # 8 local row tiles
NG = NCORES * M           # 8192 gathered rows
CHUNK = 2048              # sim free-dim chunk (4-bank PSUM tile)
NCHUNK = NG // CHUNK      # 4 sim chunks per row tile

# Schraudolph integer exp, int16/bf16 variant: the bf16 bit pattern of
# exp(x) is approximately int16(A16*x + B16); the bf16 output gets the DVE
# 2x reduce rate. B16 offset tuned for minimal loss bias.
SCH_A16 = float(2.0**23 / np.log(2.0) / 65536.0)
SCH_MUL = SCH_A16 * SCALE
SCH_ADD = float((16256.0 - 4.0) - SCH_A16 * SCALE)
SCHRAU = os.environ.get("KERNEL_SCHRAU", "1") == "1"

SKIP = set(os.environ.get("KERNEL_SKIP", "").split(",")) - {""}


def _build():
    nc = bacc.Bacc(num_devices=NCORES)

    x = nc.dram_tensor("x", [128, KT, M], F8, kind="ExternalInput")
    w0 = nc.dram_tensor("w0", [NT, 128, KT, 128], F8, kind="ExternalInput")
    w1 = nc.dram_tensor("w1", [NT, 128, KT, 128], F8, kind="ExternalInput")
    w2 = nc.dram_tensor("w2", [128, KT, 128], BF, kind="ExternalInput")
    b0 = nc.dram_tensor("b0", [128, NT], F32, kind="ExternalInput")
    b1 = nc.dram_tensor("b1", [128, NT], F32, kind="ExternalInput")
    b2 = nc.dram_tensor("b2", [128, 1], F32, kind="ExternalInput")
    # raw per-row stats [stot | dself | dpos]; the cheap final
    # log/exp/combine runs on the host in f64
    out = nc.dram_tensor("out", [128, 3 * MT], F32, kind="ExternalOutput")

    AF = mybir.ActivationFunctionType
    MULT = mybir.AluOpType.mult
    ADD = mybir.AluOpType.add
    DR = mybir.MatmulPerfMode.DoubleRow

    with tile.TileContext(nc) as tc:
        with (
            tc.tile_pool(name="singles", bufs=1) as singles,
            tc.tile_pool(name="small", bufs=4) as small,
            tc.tile_pool(name="esc", bufs=3) as esc,
            tc.tile_pool(name="pmm", bufs=2, space="PSUM") as pmm,
            tc.tile_pool(name="dram", bufs=1, space="DRAM") as dram,
        ):
            iss = [nc.sync, nc.scalar, nc.gpsimd]

            # ---- pinned weight slabs; first L0 slabs lead the queue ----
            w0s = []
            for tn in range(NT):
                ws = singles.tile([128, KT, 128], F8, name=f"w0s{tn}")
                w0s.append(ws)
            w1s = []
            for tn in range(NT):
                ws = singles.tile([128, KT, 128], F8, name=f"w1s{tn}")
                w1s.append(ws)

            # issue weight/input DMAs in first-use order, round-robin across
            # the three posting queues: the first L0 chain blocks on w0s[0] +
            # x[k0,k1]; w0s[tn] is needed at ~23+4.3*(tn//4) us; w1s[tn] at
            # ~41+1.3*tn us
            a_x = singles.tile([128, KT, M], F8)
            wsl2 = singles.tile([128, KT, 128], BF)
            order = [("w0", 0), ("x", 0), ("x", 1)]
            order += [("x", tk) for tk in range(2, 6)]
            order += [("w0", 1)]
            order += [("x", tk) for tk in range(6, 10)]
            order += [("w0", 2)]
            order += [("x", tk) for tk in range(10, 14)]
            order += [("w0", 3), ("x", 14), ("x", 15)]
            order += [("w0", tn) for tn in range(4, NT)]
            order += [("w1", tn) for tn in range(NT)]
            order += [("w2", 0)]
            for i, (kind, j) in enumerate(order):
                eng = iss[i % 3]
                if kind == "x":
                    eng.dma_start(out=a_x[:, j, :], in_=x[:, j, :])
                elif kind == "w0":
                    eng.dma_start(out=w0s[j], in_=w0[j])
                elif kind == "w1":
                    eng.dma_start(out=w1s[j], in_=w1[j])
                else:
                    eng.dma_start(out=wsl2, in_=w2[:, :, :])

            ident = singles.tile([128, 128], F32)
            make_identity(nc, ident)
            b0s = singles.tile([128, NT], F32)
            b1s = singles.tile([128, NT], F32)
            b2s = singles.tile([128, 1], F32)
            nc.scalar.dma_start(out=b0s, in_=b0[:, :])
            nc.scalar.dma_start(out=b1s, in_=b1[:, :])
            nc.scalar.dma_start(out=b2s, in_=b2[:, :])
            ones = singles.tile([128, 128], F32)
            nc.vector.memset(ones, 1.0)
            nbias = singles.tile([128, 1], F32)
            nc.vector.memset(nbias, -SCALE)
            pbias = singles.tile([128, 1], F32)
            nc.vector.memset(pbias, SCALE)

            h0 = singles.tile([128, NT, M], F8)
            h1 = singles.tile([128, NT, M], BF)
            eT = singles.tile([128, M], F32)
            sq = singles.tile([128, M], F32)
            rnorm = singles.tile([128, M], F32)
            rrec = singles.tile([128, M], F32)
            fT = singles.tile([128, M], BF)
            FT = singles.tile([128, NG], BF)

            def hidden_layer(src, dst, weights, bias_s, func, hsl):
                """fp8 DoubleRow layer on one 512-row half; 4 tn chains per
                PSUM tile, 4 512-col ACT drains."""
                for tn0 in range(0, NT, 4):
                    ps = pmm.tile([128, CHUNK], F32, tag="mm")
                    for j in range(4):
                        tn = tn0 + j
                        for tk in range(0, KT, 2):
                            nc.tensor.matmul(
                                ps[:, j * 512 : (j + 1) * 512],
                                lhsT=weights[tn][:, tk : tk + 2, :],
                                rhs=src[:, tk : tk + 2, hsl],
                                start=(tk == 0),
                                stop=(tk == KT - 2),
                                perf_mode=DR,
                            )
                    for j in range(4):
                        tn = tn0 + j
                        nc.scalar.activation(
                            out=dst[:, tn, hsl],
                            in_=ps[:, j * 512 : (j + 1) * 512],
                            func=func,
                            bias=bias_s[:, tn : tn + 1],
                            scale=1.0,
                        )

            def l2_norm_gather(csl, ft_off, tag):
                """Layer 2 + normalize + all-gather one row-group; the
                gathered blocks land at FT[:, ft_off:ft_off+8*width]."""
                width = csl.stop - csl.start
                ps2 = pmm.tile([128, CHUNK], F32, tag="mm")
                for tk in range(KT):
                    nc.tensor.matmul(
                        ps2[:, 0:width],
                        lhsT=wsl2[:, tk, :],
                        rhs=h1[:, tk, csl],
                        start=(tk == 0),
                        stop=(tk == KT - 1),
                    )
                nc.scalar.activation(
                    out=eT[:, csl], in_=ps2[:, 0:width], func=AF.Identity,
                    bias=b2s[:, 0:1], scale=1.0,
                )
                nc.vector.tensor_mul(sq[:, csl], eT[:, csl], eT[:, csl])
                nc.tensor.matmul(
                    ps2[:, width : 2 * width], lhsT=ones, rhs=sq[:, csl],
                    start=True, stop=True,
                )
                nc.scalar.activation(
                    out=rnorm[:, csl], in_=ps2[:, width : 2 * width], func=AF.Sqrt,
                    scale=1.0,
                )
                nc.vector.reciprocal_approx_fast(
                    out=rrec[:, csl], in_=rnorm[:, csl]
                )
                nc.vector.tensor_mul(fT[:, csl], eT[:, csl], rrec[:, csl])

                cc_in = dram.tile([128, width], BF, name=f"cc_in{tag}")
                cc_out = dram.tile([NCORES * 128, width], BF, name=f"cc_out{tag}")
                nc.sync.dma_start(out=cc_in, in_=fT[:, csl])
                if "collective" in SKIP:
                    for r in range(NCORES):
                        nc.sync.dma_start(
                            out=cc_out[r * 128 : (r + 1) * 128, :], in_=cc_in[:, :]
                        )
                else:
                    nc.gpsimd.collective_compute(
                        "AllGather",
                        mybir.AluOpType.bypass,
                        replica_groups=[list(range(NCORES))],
                        ins=[cc_in.opt()],
                        outs=[cc_out.opt()],
                    )
                nc.sync.dma_start(
                    out=FT[:, ft_off : ft_off + NCORES * width],
                    in_=cc_out[:, :].rearrange("(r p) w -> p r w", r=NCORES),
                )

            # FT layout: [half-A blocks (4096) | Q3 blocks (2048) | Q4 (2048)]
            # Half B gathers as two quarters so the two mesh collectives
            # overlap and the final one carries only 64KB/core.
            # Both L0 halves run before L1 so the first w1 slab isn't needed
            # until ~75us into the HBM-limited weight stream.
            hidden_layer(a_x, h0, w0s, b0s, AF.Relu, slice(0, HM))
            hidden_layer(a_x, h0, w0s, b0s, AF.Relu, slice(HM, M))
            hidden_layer(h0, h1, w1s, b1s, AF.Identity, slice(0, HM))
            l2_norm_gather(slice(0, HM), 0, "A")
            hidden_layer(h0, h1, w1s, b1s, AF.Identity, slice(HM, M))
            l2_norm_gather(slice(HM, HM + HM // 2), NG // 2, "Q3")
            l2_norm_gather(slice(HM + HM // 2, M), 3 * NG // 4, "Q4")

            # ---- self/pos diagonals from local features ----
            dself_all = singles.tile([128, MT], F32)
            dpos_all = singles.tile([128, MT], F32)
            for m in range(MT):
                pm = (m + MT // 2) % MT
                lhs = fT[:, m * 128 : (m + 1) * 128]
                psd = pmm.tile([128, CHUNK], F32, tag="mm")
                nc.tensor.matmul(
                    psd[:, 0:128], lhsT=lhs, rhs=fT[:, m * 128 : (m + 1) * 128],
                    start=True, stop=True,
                )
                nc.tensor.matmul(
                    psd[:, 128:256], lhsT=lhs, rhs=fT[:, pm * 128 : (pm + 1) * 128],
                    start=True, stop=True,
                )
                dsc = small.tile([128, 128], F32, tag="dscratch")
                nc.vector.tensor_mul(dsc, psd[:, 0:128], ident)
                nc.vector.reduce_sum(
                    dself_all[:, m : m + 1], dsc, axis=mybir.AxisListType.X
                )
                dsc2 = small.tile([128, 128], F32, tag="dscratch")
                nc.vector.tensor_mul(dsc2, psd[:, 128:256], ident)
                nc.vector.reduce_sum(
                    dpos_all[:, m : m + 1], dsc2, axis=mybir.AxisListType.X
                )

            # ship the diagonals while the sim phase runs
            nc.scalar.dma_start(out=out[:, MT : 2 * MT], in_=dself_all)
            nc.scalar.dma_start(out=out[:, 2 * MT : 3 * MT], in_=dpos_all)

            # ---- sim + exp-sum; Exp+accum on ACT, Schraudolph on DVE ----
            outv = singles.tile([128, MT], F32)
            stot_all = singles.tile([128, MT], F32)
            sums = singles.tile([128, MT, NCHUNK], F32)
            if "phase3" in SKIP:
                nc.vector.tensor_copy(outv, fT[:, :MT])
            # c-outer order: chunks over the first-half columns (c 0,1)
            # depend only on the first gather and fill the second gather's
            # latency window.
            for c, m in ([] if "phase3" in SKIP else
                         [(c, m) for c in range(NCHUNK) for m in range(MT)]):
                idx = c * MT + m
                lhs = fT[:, m * 128 : (m + 1) * 128]
                ps = pmm.tile([128, CHUNK], F32, tag="mm")
                for q in range(CHUNK // 512):
                    j0 = c * CHUNK + q * 512
                    nc.tensor.matmul(
                        ps[:, q * 512 : (q + 1) * 512],
                        lhsT=lhs, rhs=FT[:, j0 : j0 + 512],
                        start=True, stop=True,
                    )
                if SCHRAU and idx % 3 == 2:
                    sch = esc.tile([128, CHUNK], mybir.dt.int16, tag="sch")
                    nc.vector.tensor_scalar(
                        out=sch, in0=ps, scalar1=SCH_MUL, scalar2=SCH_ADD,
                        op0=MULT, op1=ADD,
                    )
                    nc.vector.reduce_sum(
                        sums[:, m, c : c + 1], sch.bitcast(BF),
                        axis=mybir.AxisListType.X,
                    )
                else:
                    escr = esc.tile([128, CHUNK], BF, tag="escr")
                    nc.scalar.activation(
                        out=escr, in_=ps, func=AF.Exp, scale=SCALE,
                        bias=nbias, accum_out=sums[:, m, c : c + 1],
                    )
            if "phase3" not in SKIP:
                nc.vector.reduce_sum(
                    stot_all, sums, axis=mybir.AxisListType.X
                )

            if "phase3" in SKIP:
                nc.sync.dma_start(out=out[:, 0:MT], in_=outv)
            else:
                nc.sync.dma_start(out=out[:, 0:MT], in_=stot_all)

    nc.finalize()
    return nc


_NC_CACHE = None


def _get_nc():
    global _NC_CACHE
    if _NC_CACHE is None:
        _NC_CACHE = _build()
    return _NC_CACHE


def host_reduce(o):
    """Sum of per-row losses from one core's raw [stot | dself | dpos] out."""
    o = np.asarray(o, np.float64)
    stot, dself, dpos = o[:, 0:MT], o[:, MT : 2 * MT], o[:, 2 * MT : 3 * MT]
    sexcl = stot - np.exp(SCALE * dself - SCALE)
    return (np.log(sexcl) + SCALE * (1.0 - dpos)).sum()


def _prep_w(W, ntiles, dt=BF16):
    K = W.shape[0]
    kt = K // 128
    arr = W.reshape(kt, 128, ntiles, 128).transpose(2, 1, 0, 3)
    return np.ascontiguousarray(arr.astype(dt))


def _prep_b(b, ntiles):
    return np.ascontiguousarray(
        np.asarray(b, np.float32).reshape(ntiles, 128).T
    )


def kernel(input1, input2, W0, b0, W1, b1, W2, b2):
    input1 = np.asarray(input1, np.float32)
    input2 = np.asarray(input2, np.float32)
    w0p = _prep_w(np.asarray(W0, np.float32), NT, FP8)
    w1p = _prep_w(np.asarray(W1, np.float32), NT, FP8)
    w2p = _prep_w(np.asarray(W2, np.float32), 1)[0]
    b0p = _prep_b(b0, NT)
    b1p = _prep_b(b1, NT)
    b2p = np.ascontiguousarray(np.asarray(b2, np.float32).reshape(128, 1))

    in_maps = []
    for r in range(NCORES):
        xr = np.concatenate(
            [input1[r * BS : (r + 1) * BS], input2[r * BS : (r + 1) * BS]], axis=0
        )
        xp = np.ascontiguousarray(
            xr.reshape(M, KT, 128).transpose(2, 1, 0).astype(FP8)
        )
        in_maps.append(
            {
                "x": xp, "w0": w0p, "w1": w1p, "w2": w2p,
                "b0": b0p, "b1": b1p, "b2": b2p,
            }
        )

    nc = _get_nc()
    res = run_bass_kernel_spmd(
        nc,
        in_maps,
        core_ids=list(range(NCORES)),
        trace=bool(int(os.environ.get("KERNEL_TRACE", "0"))),
    )
    total = np.float64(0.0)
    for r in range(NCORES):
        total += host_reduce(np.asarray(res.results[r]["out"], np.float64))
    loss = np.float32(total / (2 * B))
    if res.exec_time_ns is not None:
        kernel.last_exec_time_ns = res.exec_time_ns
    return np.asarray(loss, np.float32)


kernel.last_exec_time_ns = None
